# revision 27
# baseline (speedup 1.0000x reference)
"""CCX loss kernel for Trainium2 (8 NeuronCores, data-parallel over batch).

Math (per batch element n, with C=256 channels, HW=64*64=4096 pixels):
  y_mu[c]   = mean over (n, h, w) of y            (host, tiny)
  x_c = x - y_mu ; y_c = y - y_mu                 (device)
  x_n = x_c/||x_c||_C ; y_n = y_c/||y_c||_C       (device)
  s[i,j]    = sum_c x_n[c,i] y_n[c,j]             (device matmul, f32r)
  d = 1-s ; dt = d/(dmin_i+eps) ; w = exp((1-dt)/0.5)
  ccx_ij = w/sum_j w ; ccx_n = mean_j max_i ccx_ij
  loss = mean_n -log(ccx_n + eps)                 (host, 8 scalars)

Key identities used on device:
  w_ij = exp(s*a_i + b_i),  a_i = 2/(dmin_i+eps), b_i = 2-a_i
  s*a_i = G*alpha_i with G = x_c^T y_n (unnormalized-x matmul),
          alpha_i = a_i/||x_c[:,i]||
  max_i ccx_ij = exp(max_i (G^T[j,i]*alpha_i + (b_i - lnZ_i)))
  -> pass 2: K=1 ones matmul seeds psum with the bias row b2, the
     channel matmuls accumulate G^T*alpha, one reduce_max per tile.

Perf structure (TRN2):
  - x/y stream in per-K-group halves through small staging tiles;
    centering trails each half (overlaps DMA).
  - pass1 per block: matmuls (K-group outer, shared LDWEIGHTS),
    512-wide partial row maxes trailing the matmul stream (short
    serial tail), all-DVE stats chain, exp in-place + Z accumulation.
  - pass2: bias seed + channel matmuls + reduce_max per tile; the
    channel matmuls optionally run in fp8e4 DoubleRow (2 K-groups in
    one pass at 0.5 cyc/row) with BASS_FP8=1.
"""

import os
import sys

import numpy as np

sys.path.insert(0, "/opt/trn_rl_repo")
os.environ.setdefault("JAX_PLATFORMS", "axon")

import concourse.mybir as mybir
import concourse.tile as tile
from concourse import bacc, bass_isa
import concourse.bass_utils as _bass_utils
from concourse.bass_utils import run_bass_kernel_spmd

if os.environ.get("BASS_LDW_OPT", "0") == "1":
    _orig_run_command = _bass_utils.run_command

    def _run_command_ldwopt(cmd, *a, **kw):
        cmd = ["--enable-ldw-opt=true" if c == "--enable-ldw-opt=false" else c
               for c in cmd]
        return _orig_run_command(cmd, *a, **kw)

    _bass_utils.run_command = _run_command_ldwopt

N, C, H, W = 8, 256, 64, 64
HW = H * W          # 4096
EPS = 1e-6
F32 = mybir.dt.float32
F32R = mybir.dt.float32r
BF16 = mybir.dt.bfloat16
FP8 = mybir.dt.float8e4
ALU = mybir.AluOpType
ACTF = mybir.ActivationFunctionType

TW = int(os.environ.get("BASS_TW", "2048"))    # psum tile width
NTB = HW // TW                                  # tiles per block row
NBUF = 16384 // (TW * 4)                        # psum ring generations
NB = HW // 128                                  # 32 row/col blocks
NS = TW // 512                                  # 512-slices per tile
USE_FP8 = os.environ.get("BASS_FP8", "0") == "1"
USE_DVEMAX = os.environ.get("BASS_DVEMAX", "0") == "1"

if USE_DVEMAX:
    # Register a custom DVE op: out = in0 + in1, accum_out = max(c1, max out)
    # (fused bias-add + max-reduce; replaces the K=1 bias matmul + the
    # plain reduce_max in pass 2).
    import numpy as _np
    import concourse.dve_ops as _dve_ops
    from concourse.dve_spec import Spec as _Spec, Src0 as _Src0, Src1 as _Src1
    from concourse.dve_spec import C1 as _C1, lower as _dve_lower
    from concourse.dve_spec import maxx as _maxx, _has_src1 as _dve_has_src1
    from concourse.dve_uop import DveOpSpec as _DveOpSpec

    if "ADD_MAX_REDUCE" not in _dve_ops._SUB_OPCODE_FOR_NAME:
        _row = _dve_ops._CUSTOM_DVE_ROW_BASE + len(_dve_ops.OPS)
        assert _row < 0x20

        def _amr_ref(in0, in1, c0, c1, c2):
            b = (in0.astype(_np.float32) + in1).astype(_np.float32)
            acc = _np.maximum(
                b.reshape(b.shape[0], -1).max(axis=-1, keepdims=True), c1)
            return b, acc

        _spec = _Spec(body=_Src0 + _Src1, accum=_maxx, accum_init=_C1,
                      reference=_amr_ref)
        _shas = {}
        for _ver in ("v3",):
            _lowered = _DveOpSpec(
                name="ADD_MAX_REDUCE", opcode=_row,
                uops=_dve_lower(_spec, ver=_ver),
                rd1_en=_dve_has_src1(_spec))
            _shas[_ver] = _lowered.sha(_ver)
        _AMR = _dve_ops.DveOp(
            "ADD_MAX_REDUCE", _spec, subdim=False, uops_sha=_shas)
        _dve_ops.OPS.append(_AMR)
        _dve_ops.CUSTOM_DVE_SPECS["ADD_MAX_REDUCE"] = _spec
        _dve_ops._SUB_OPCODE_FOR_NAME["ADD_MAX_REDUCE"] = _row
    else:
        _AMR = next(o for o in _dve_ops.OPS if o.name == "ADD_MAX_REDUCE")

_cached = {}


def _build():
    nc = bacc.Bacc(None, target_bir_lowering=False, debug=True)
    xs = nc.dram_tensor("xs", [C, HW], F32, kind="ExternalInput")
    ys = nc.dram_tensor("ys", [C, HW], F32, kind="ExternalInput")
    ymu = nc.dram_tensor("ymu", [128, 2], F32, kind="ExternalInput")
    out = nc.dram_tensor("out", [1, 1], F32, kind="ExternalOutput")
    scr_nx = nc.dram_tensor("scr_nx", [1, HW], F32)  # x normsq row
    scr_ny = nc.dram_tensor("scr_ny", [1, HW], F32)  # y normsq row
    scr_y = nc.dram_tensor("scr_y", [NB, 128], F32)
    scr_a = nc.dram_tensor("scr_a", [NB, 128], F32)
    scr_b = nc.dram_tensor("scr_b", [NB, 128], F32)

    import concourse.bass as bass_mod

    with tile.TileContext(nc) as tc:
        with (
            tc.tile_pool(name="big", bufs=1) as big,
            tc.tile_pool(name="stage", bufs=2) as stg,
            tc.tile_pool(name="bc3", bufs=1) as bc3,
            tc.tile_pool(name="sq", bufs=2) as sqp,
            tc.tile_pool(name="small", bufs=1) as sm,
            tc.tile_pool(name="mmq", bufs=NBUF, space="PSUM") as mmq,
        ):
            xc = big.tile([128, 2, HW], BF16, tag="xc")
            yn = big.tile([128, 2, HW], BF16, tag="yn")
            ymu_sb = sm.tile([128, 2], F32, tag="ymu")
            nc.sync.dma_start(out=ymu_sb[:, :], in_=ymu[:, :])

            ones_col = sm.tile([128, 1], BF16, tag="ones_col")
            nc.vector.memset(ones_col[:, :], 1.0)
            negymu = sm.tile([128, 2], F32, tag="negymu")
            nc.vector.tensor_scalar(
                out=negymu[:, :], in0=ymu_sb[:, :], scalar1=-1.0,
                scalar2=None, op0=ALU.mult)
            ones_row = sm.tile([1, 128], F32R, tag="ones_row")
            ones_row_f = sm.tile([1, 128], F32, tag="ones_row_f")
            nc.vector.memset(ones_row_f[:, :], 1.0)
            nc.vector.tensor_scalar(
                out=ones_row[:, :], in0=ones_row_f[:, :], scalar1=1.0,
                scalar2=None, op0=ALU.mult)

            # ---------------- staged loads + centering --------------------
            # y halves on SP queue + ACT centering; x halves on gpsimd
            # queue + DVE centering.  Staging ring overlaps DMA/compute.
            ystg = [stg.tile([128, HW], F32, tag="stage", name=f"ys{g}")
                    for g in range(2)]
            for g in range(2):
                nc.sync.dma_start(
                    out=ystg[g][:, :],
                    in_=ys.rearrange("(g p) j -> g p j", p=128)[g])
            xstg = [stg.tile([128, HW], F32, tag="stage", name=f"xs{g}")
                    for g in range(2)]
            for g in range(2):
                nc.gpsimd.dma_start(
                    out=xstg[g][:, :],
                    in_=xs.rearrange("(g p) j -> g p j", p=128)[g])
            for g in range(2):
                nc.scalar.activation(
                    out=yn[:, g, :], in_=ystg[g][:, :],
                    func=ACTF.Identity, bias=negymu[:, g : g + 1], scale=1.0)
            for g in range(2):
                nc.vector.tensor_scalar(
                    out=xc[:, g, :], in0=xstg[g][:, :],
                    scalar1=ymu_sb[:, g : g + 1], scalar2=None,
                    op0=ALU.subtract)

            # ---------------- sumsq rows -> invy/invx ---------------------
            nrow = sm.tile([1, HW], F32, tag="nrow")

            def sumsq_rows(src, scr, use_act):
                sqs = []
                for g in range(2):
                    sq = sqp.tile([128, HW], BF16, tag="sqt")
                    if use_act:
                        nc.scalar.activation(
                            out=sq[:, :], in_=src[:, g, :],
                            func=ACTF.Square)
                    else:
                        nc.vector.tensor_tensor(
                            out=sq[:, :], in0=src[:, g, :],
                            in1=src[:, g, :], op=ALU.mult)
                    sqs.append(sq)
                for h in range(HW // TW):
                    pr = mmq.tile([1, TW], F32, tag="pq",
                                  name=f"pr_{scr.name}_{h}")
                    for g in range(2):
                        for s in range(NS):
                            j0 = TW * h + 512 * s
                            nc.tensor.matmul(
                                pr[:, 512 * s : 512 * (s + 1)],
                                ones_col[:, :],
                                sqs[g][:, j0 : j0 + 512],
                                start=(g == 0), stop=(g == 1))
                    nc.scalar.activation(
                        out=nrow[0:1, TW * h : TW * (h + 1)],
                        in_=pr[0:1, :], func=ACTF.Copy)
                nc.sync.dma_start(out=scr[0, :], in_=nrow[0:1, :])

            sumsq_rows(yn, scr_ny, use_act=True)
            nsqy = sm.tile([128, NB], F32, tag="nsqy")
            nc.sync.dma_start(
                out=nsqy[:, :], in_=scr_ny[0, :].rearrange("(r p) -> p r", p=128))
            normy = sm.tile([128, NB], F32, tag="normy")
            nc.scalar.activation(out=normy[:, :], in_=nsqy[:, :], func=ACTF.Sqrt)
            invy = sm.tile([128, NB], F32, tag="invy")
            nc.vector.reciprocal(invy[:, :], normy[:, :])
            nc.sync.dma_start(
                out=scr_y[:, :].rearrange("r p -> p r"), in_=invy[:, :])
            invybc = bc3.tile([128, HW], F32, tag="bcast")
            bcast_src_y = bass_mod.AP(
                tensor=scr_y[:, :].tensor, offset=0, ap=[[0, 128], [1, HW]])
            nc.sync.dma_start(out=invybc[:, :], in_=bcast_src_y)

            # y_n = y_c * invy (in place, f32r)
            for g in range(2):
                nc.vector.tensor_tensor(
                    out=yn[:, g, :], in0=yn[:, g, :],
                    in1=invybc[:, :], op=ALU.mult)

            # x norms (overlaps pass1 head; only needed by the stats chain)
            sumsq_rows(xc, scr_nx, use_act=True)
            nsqx = sm.tile([128, NB], F32, tag="nsqx")
            nc.sync.dma_start(
                out=nsqx[:, :], in_=scr_nx[0, :].rearrange("(r p) -> p r", p=128))
            normx = sm.tile([128, NB], F32, tag="normx")
            nc.scalar.activation(out=normx[:, :], in_=nsqx[:, :], func=ACTF.Sqrt)
            invx = sm.tile([128, NB], F32, tag="invx")
            nc.vector.reciprocal(invx[:, :], normx[:, :])
            invx2 = sm.tile([128, NB], F32, tag="invx2")     # 2*invx
            nc.vector.tensor_scalar(
                out=invx2[:, :], in0=invx[:, :], scalar1=2.0,
                scalar2=None, op0=ALU.mult)
            ninvx = sm.tile([128, NB], F32, tag="ninvx")     # -invx
            nc.vector.tensor_scalar(
                out=ninvx[:, :], in0=invx[:, :], scalar1=-1.0,
                scalar2=None, op0=ALU.mult)

            # ---------------- PASS 1: row max + Z -------------------------
            gacc = sm.tile([128, NB * NTB], F32, tag="gacc")
            zacc = sm.tile([128, NB * NTB], F32, tag="zacc")
            gmaxc = sm.tile([128, NB], F32, tag="gmaxc")
            reccol = sm.tile([128, NB], F32, tag="reccol")
            tmpc = sm.tile([128, NB], F32, tag="tmpc")
            ab2 = sm.tile([128, 2 * NB], F32, tag="ab2")  # alpha | b2
            bcol = sm.tile([128, NB], F32, tag="bcol")
            zsum = sm.tile([128, NB], F32, tag="zsum")
            lnz = sm.tile([128, NB], F32, tag="lnz")

            for r in range(NB):
                pqs = [mmq.tile([128, TW], F32, tag="pq", name=f"p1_{r}_{t}")
                       for t in range(NTB)]
                for t in range(NTB):
                    for g in range(2):
                        for s in range(NS):
                            j0 = TW * t + 512 * s
                            nc.tensor.matmul(
                                pqs[t][:, 512 * s : 512 * (s + 1)],
                                xc[:, g, 128 * r : 128 * (r + 1)],
                                yn[:, g, j0 : j0 + 512],
                                start=(g == 0), stop=(g == 1))
                    nc.vector.reduce_max(
                        gacc[:, NTB * r + t : NTB * r + t + 1],
                        pqs[t][:, :], axis=mybir.AxisListType.X)
                # all-DVE row stats chain (back-to-back, same engine)
                nc.vector.reduce_max(
                    gmaxc[:, r : r + 1],
                    gacc[:, NTB * r : NTB * (r + 1)],
                    axis=mybir.AxisListType.X)
                nc.vector.tensor_scalar(
                    out=tmpc[:, r : r + 1], in0=gmaxc[:, r : r + 1],
                    scalar1=ninvx[:, r : r + 1], scalar2=float(1.0 + EPS),
                    op0=ALU.mult, op1=ALU.add)
                nc.vector.reciprocal(reccol[:, r : r + 1], tmpc[:, r : r + 1])
                nc.vector.tensor_scalar(
                    out=ab2[:, r : r + 1], in0=reccol[:, r : r + 1],
                    scalar1=invx2[:, r : r + 1], scalar2=None, op0=ALU.mult)
                nc.vector.tensor_scalar(
                    out=bcol[:, r : r + 1], in0=reccol[:, r : r + 1],
                    scalar1=-2.0, scalar2=2.0, op0=ALU.mult, op1=ALU.add)
                for t in range(NTB):
                    nc.scalar.activation(
                        out=pqs[t][:, :], in_=pqs[t][:, :], func=ACTF.Exp,
                        bias=bcol[:, r : r + 1],
                        scale=ab2[:, r : r + 1],
                        accum_out=zacc[:, NTB * r + t : NTB * r + t + 1])

            # ---------------- interlude: b2 = b - lnZ; broadcasts ---------
            nc.vector.reduce_sum(
                zsum[:, :],
                zacc[:, :].rearrange("p (r q) -> p r q", q=NTB),
                axis=mybir.AxisListType.X)
            nc.scalar.activation(out=lnz[:, :], in_=zsum[:, :], func=ACTF.Ln)
            nc.vector.tensor_tensor(
                out=ab2[:, NB : 2 * NB], in0=bcol[:, :], in1=lnz[:, :],
                op=ALU.subtract)

            # alpha -> bcast via SP queue; b2 -> row via gpsimd queue
            nc.sync.dma_start(
                out=scr_a[:, :].rearrange("r p -> p r"), in_=ab2[:, 0:NB])
            nc.gpsimd.dma_start(
                out=scr_b[:, :].rearrange("r p -> p r"),
                in_=ab2[:, NB : 2 * NB])
            abc = bc3.tile([128, HW], F32, tag="bcast")
            bcast_src_a = bass_mod.AP(
                tensor=scr_a[:, :].tensor, offset=0, ap=[[0, 128], [1, HW]])
            nc.sync.dma_start(out=abc[:, :], in_=bcast_src_a)
            if USE_DVEMAX:
                b2bc = bc3.tile([128, HW], F32, tag="bcast2")
                bcast_src_b = bass_mod.AP(
                    tensor=scr_b[:, :].tensor, offset=0,
                    ap=[[0, 128], [1, HW]])
                nc.gpsimd.dma_start(out=b2bc[:, :], in_=bcast_src_b)
                b2row = None
            else:
                nc.gpsimd.dma_start(
                    out=nrow[0:1, :],
                    in_=scr_b[:, :].rearrange("r p -> (r p)"))
                b2row = sm.tile([1, HW], F32R, tag="b2row")
                nc.vector.tensor_scalar(
                    out=b2row[:, :], in0=nrow[0:1, :], scalar1=1.0,
                    scalar2=None, op0=ALU.mult)

            # x2 = x_c * alpha (in place, f32r), chunked so pass2 can start
            XCH = 1024
            for j0 in range(0, HW, XCH):
                for g in range(2):
                    nc.vector.tensor_tensor(
                        out=xc[:, g, j0 : j0 + XCH],
                        in0=xc[:, g, j0 : j0 + XCH],
                        in1=abc[:, j0 : j0 + XCH], op=ALU.mult)

            # optional fp8 copies for pass2 DoubleRow matmuls
            if USE_FP8:
                x2f8 = stg.tile([128, 2, HW], FP8, tag="f8", name="x2f8")
                ynf8 = stg.tile([128, 2, HW], FP8, tag="f8", name="ynf8")
                for g in range(2):
                    nc.vector.tensor_scalar(
                        out=x2f8[:, g, :], in0=xc[:, g, :],
                        scalar1=1.0, scalar2=None, op0=ALU.mult)
                    nc.scalar.activation(
                        out=ynf8[:, g, :], in_=yn[:, g, :],
                        func=ACTF.Copy)

            # ---------------- PASS 2: bias seed + col max -----------------
            macc = sm.tile([128, NB * NTB], F32, tag="macc")
            dummy = sm.tile([128, 1], F32, tag="dummy")
            for rb in range(NB):
                pqs = [mmq.tile([128, TW], F32, tag="pq", name=f"p2_{rb}_{t}")
                       for t in range(NTB)]
                for t in range(NTB):
                    if not USE_DVEMAX:
                        for s in range(NS):
                            j0 = TW * t + 512 * s
                            nc.tensor.matmul(
                                pqs[t][:, 512 * s : 512 * (s + 1)],
                                ones_row[:, :],
                                b2row[:, j0 : j0 + 512],
                                start=True, stop=False)
                    if USE_FP8:
                        for s in range(NS):
                            j0 = TW * t + 512 * s
                            nc.tensor.matmul(
                                pqs[t][:, 512 * s : 512 * (s + 1)],
                                ynf8[:, :, 128 * rb : 128 * (rb + 1)],
                                x2f8[:, :, j0 : j0 + 512],
                                start=False, stop=True,
                                perf_mode=mybir.MatmulPerfMode.DoubleRow)
                    else:
                        for g in range(2):
                            for s in range(NS):
                                j0 = TW * t + 512 * s
                                nc.tensor.matmul(
                                    pqs[t][:, 512 * s : 512 * (s + 1)],
                                    yn[:, g, 128 * rb : 128 * (rb + 1)],
                                    xc[:, g, j0 : j0 + 512],
                                    start=(USE_DVEMAX and g == 0),
                                    stop=(g == 1))
                    if USE_DVEMAX:
                        nc.vector._custom_dve(
                            _AMR,
                            out=dummy[:, 0:1].broadcast_to((128, TW)),
                            in0=pqs[t][:, :],
                            in1=b2bc[:, TW * t : TW * (t + 1)],
                            s1=-1e30,
                            accum_out=macc[:, NTB * rb + t : NTB * rb + t + 1])
                    else:
                        nc.vector.reduce_max(
                            macc[:, NTB * rb + t : NTB * rb + t + 1],
                            pqs[t][:, :], axis=mybir.AxisListType.X)

            # ---------------- final ---------------------------------------
            mcol = sm.tile([128, NB], F32, tag="mcol")
            nc.vector.reduce_max(
                mcol[:, :],
                macc[:, :].rearrange("p (r q) -> p r q", q=NTB),
                axis=mybir.AxisListType.X)
            expm = sm.tile([128, NB], F32, tag="expm")
            csum = sm.tile([128, 1], F32, tag="csum")
            nc.scalar.activation(
                out=expm[:, :], in_=mcol[:, :], func=ACTF.Exp,
                accum_out=csum[:, :])
            tot = sm.tile([128, 1], F32, tag="tot")
            nc.gpsimd.partition_all_reduce(
                tot[:, :], csum[:, :], channels=128,
                reduce_op=bass_isa.ReduceOp.add)
            res = sm.tile([1, 1], F32, tag="res")
            nc.vector.tensor_scalar(
                out=res[:, :], in0=tot[0:1, :], scalar1=float(1.0 / HW),
                scalar2=None, op0=ALU.mult)
            nc.sync.dma_start(out=out[:, :], in_=res[:, :])
    nc.compile()
    return nc


def _get_nc():
    if "nc" not in _cached:
        _cached["nc"] = _build()
    return _cached["nc"]


def run_device(x, y, trace=False):
    """x, y: (N, C, H, W) float32. Returns (ccx (N,), BassKernelResults)."""
    x = np.ascontiguousarray(np.asarray(x, dtype=np.float32))
    y = np.ascontiguousarray(np.asarray(y, dtype=np.float32))
    ymu = y.mean(axis=(0, 2, 3), dtype=np.float64).astype(np.float32)  # (C,)
    ymu_arr = np.ascontiguousarray(ymu.reshape(2, 128).T)  # (128, 2)
    in_maps = []
    for n in range(N):
        in_maps.append({
            "xs": np.ascontiguousarray(x[n].reshape(C, HW)),
            "ys": np.ascontiguousarray(y[n].reshape(C, HW)),
            "ymu": ymu_arr,
        })
    nc = _get_nc()
    res = run_bass_kernel_spmd(nc, in_maps, core_ids=list(range(N)), trace=trace)
    ccx = np.array([res.results[n]["out"][0, 0] for n in range(N)], dtype=np.float32)
    return ccx, res


def kernel(x, y):
    ccx, _ = run_device(x, y)
    loss = float(np.mean(-np.log(ccx.astype(np.float64) + EPS)))
    return np.float32(loss)


if __name__ == "__main__":
    rng = np.random.default_rng(0)
    x = rng.standard_normal((N, C, H, W), dtype=np.float32)
    y = rng.standard_normal((N, C, H, W), dtype=np.float32)
    print("loss:", kernel(x, y))


# revision 29
# speedup vs baseline: 1.1947x; 1.1947x over previous
"""CCX loss kernel for Trainium2 (8 NeuronCores, data-parallel over batch).

Math (per batch element n, with C=256 channels, HW=64*64=4096 pixels):
  y_mu[c]   = mean over (n, h, w) of y            (host, tiny)
  x_c = x - y_mu ; y_c = y - y_mu                 (device)
  x_n = x_c/||x_c||_C ; y_n = y_c/||y_c||_C       (device)
  s[i,j]    = sum_c x_n[c,i] y_n[c,j]             (device matmul, f32r)
  d = 1-s ; dt = d/(dmin_i+eps) ; w = exp((1-dt)/0.5)
  ccx_ij = w/sum_j w ; ccx_n = mean_j max_i ccx_ij
  loss = mean_n -log(ccx_n + eps)                 (host, 8 scalars)

Key identities used on device:
  w_ij = exp(s*a_i + b_i),  a_i = 2/(dmin_i+eps), b_i = 2-a_i
  s*a_i = G*alpha_i with G = x_c^T y_n (unnormalized-x matmul),
          alpha_i = a_i/||x_c[:,i]||
  max_i ccx_ij = exp(max_i (G^T[j,i]*alpha_i + (b_i - lnZ_i)))
  -> pass 2: K=1 ones matmul seeds psum with the bias row b2, the
     channel matmuls accumulate G^T*alpha, one reduce_max per tile.

Perf structure (TRN2):
  - x/y stream in per-K-group halves through small staging tiles;
    centering trails each half (overlaps DMA).
  - pass1 per block: matmuls (K-group outer, shared LDWEIGHTS),
    512-wide partial row maxes trailing the matmul stream (short
    serial tail), all-DVE stats chain, exp in-place + Z accumulation.
  - pass2: bias seed + channel matmuls + reduce_max per tile; the
    channel matmuls optionally run in fp8e4 DoubleRow (2 K-groups in
    one pass at 0.5 cyc/row) with BASS_FP8=1.
"""

import os
import sys

import numpy as np

sys.path.insert(0, "/opt/trn_rl_repo")
os.environ.setdefault("JAX_PLATFORMS", "axon")

import concourse.mybir as mybir
import concourse.tile as tile
from concourse import bacc, bass_isa
import concourse.bass_utils as _bass_utils
from concourse.bass_utils import run_bass_kernel_spmd

if os.environ.get("BASS_LDW_OPT", "0") == "1":
    _orig_run_command = _bass_utils.run_command

    def _run_command_ldwopt(cmd, *a, **kw):
        cmd = ["--enable-ldw-opt=true" if c == "--enable-ldw-opt=false" else c
               for c in cmd]
        return _orig_run_command(cmd, *a, **kw)

    _bass_utils.run_command = _run_command_ldwopt

N, C, H, W = 8, 256, 64, 64
HW = H * W          # 4096
EPS = 1e-6
F32 = mybir.dt.float32
F32R = mybir.dt.float32r
BF16 = mybir.dt.bfloat16
FP8 = mybir.dt.float8e4
ALU = mybir.AluOpType
ACTF = mybir.ActivationFunctionType

TW = int(os.environ.get("BASS_TW", "2048"))    # psum tile width
NTB = HW // TW                                  # tiles per block row
NBUF = 16384 // (TW * 4)                        # psum ring generations
NB = HW // 128                                  # 32 row/col blocks
NS = TW // 512                                  # 512-slices per tile
USE_FP8 = os.environ.get("BASS_FP8", "0") == "1"
USE_DVEMAX = os.environ.get("BASS_DVEMAX", "0") == "1"
USE_FMAX = os.environ.get("BASS_FMAX", "1") == "1"

if USE_DVEMAX or USE_FMAX:
    # Register a custom DVE op: out = in0 + in1, accum_out = max(c1, max out)
    # (fused bias-add + max-reduce; replaces the K=1 bias matmul + the
    # plain reduce_max in pass 2).
    import numpy as _np
    import concourse.dve_ops as _dve_ops
    from concourse.dve_spec import Spec as _Spec, Src0 as _Src0, Src1 as _Src1
    from concourse.dve_spec import C1 as _C1, lower as _dve_lower
    from concourse.dve_spec import maxx as _maxx, _has_src1 as _dve_has_src1
    from concourse.dve_uop import DveOpSpec as _DveOpSpec

    if "ADD_MAX_REDUCE" not in _dve_ops._SUB_OPCODE_FOR_NAME:
        _row = _dve_ops._CUSTOM_DVE_ROW_BASE + len(_dve_ops.OPS)
        assert _row < 0x20

        def _amr_ref(in0, in1, c0, c1, c2):
            b = (in0.astype(_np.float32) + in1).astype(_np.float32)
            acc = _np.maximum(
                b.reshape(b.shape[0], -1).max(axis=-1, keepdims=True), c1)
            return b, acc

        _spec = _Spec(body=_Src0 + _Src1, accum=_maxx, accum_init=_C1,
                      reference=_amr_ref)
        _shas = {}
        for _ver in ("v3",):
            _lowered = _DveOpSpec(
                name="ADD_MAX_REDUCE", opcode=_row,
                uops=_dve_lower(_spec, ver=_ver),
                rd1_en=_dve_has_src1(_spec))
            _shas[_ver] = _lowered.sha(_ver)
        _AMR = _dve_ops.DveOp(
            "ADD_MAX_REDUCE", _spec, subdim=False, uops_sha=_shas)
        _dve_ops.OPS.append(_AMR)
        _dve_ops.CUSTOM_DVE_SPECS["ADD_MAX_REDUCE"] = _spec
        _dve_ops._SUB_OPCODE_FOR_NAME["ADD_MAX_REDUCE"] = _row
    else:
        _AMR = next(o for o in _dve_ops.OPS if o.name == "ADD_MAX_REDUCE")

_cached = {}


def _build():
    nc = bacc.Bacc(None, target_bir_lowering=False, debug=True)
    xs = nc.dram_tensor("xs", [C, HW], F32, kind="ExternalInput")
    ys = nc.dram_tensor("ys", [C, HW], F32, kind="ExternalInput")
    ymu = nc.dram_tensor("ymu", [128, 2], F32, kind="ExternalInput")
    out = nc.dram_tensor("out", [1, 1], F32, kind="ExternalOutput")
    scr_nx = nc.dram_tensor("scr_nx", [1, HW], F32)  # x normsq row
    scr_ny = nc.dram_tensor("scr_ny", [1, HW], F32)  # y normsq row
    scr_y = nc.dram_tensor("scr_y", [NB, 128], F32)
    scr_a = nc.dram_tensor("scr_a", [NB, 128], F32)
    scr_b = nc.dram_tensor("scr_b", [NB, 128], F32)

    import concourse.bass as bass_mod

    with tile.TileContext(nc) as tc:
        with (
            tc.tile_pool(name="big", bufs=1) as big,
            tc.tile_pool(name="stage", bufs=2) as stg,
            tc.tile_pool(name="bc3", bufs=1) as bc3,
            tc.tile_pool(name="sq", bufs=2) as sqp,
            tc.tile_pool(name="small", bufs=1) as sm,
            tc.tile_pool(name="mmq", bufs=NBUF, space="PSUM") as mmq,
        ):
            xc = big.tile([128, 2, HW], BF16, tag="xc")
            yn = big.tile([128, 2, HW], BF16, tag="yn")
            ymu_sb = sm.tile([128, 2], F32, tag="ymu")
            nc.sync.dma_start(out=ymu_sb[:, :], in_=ymu[:, :])

            ones_col = sm.tile([128, 1], BF16, tag="ones_col")
            nc.vector.memset(ones_col[:, :], 1.0)
            negymu = sm.tile([128, 2], F32, tag="negymu")
            nc.vector.tensor_scalar(
                out=negymu[:, :], in0=ymu_sb[:, :], scalar1=-1.0,
                scalar2=None, op0=ALU.mult)
            ones_row = sm.tile([1, 128], F32R, tag="ones_row")
            ones_row_f = sm.tile([1, 128], F32, tag="ones_row_f")
            nc.vector.memset(ones_row_f[:, :], 1.0)
            nc.vector.tensor_scalar(
                out=ones_row[:, :], in0=ones_row_f[:, :], scalar1=1.0,
                scalar2=None, op0=ALU.mult)

            # ---------------- staged loads + centering --------------------
            # y halves on SP queue + ACT centering; x halves on gpsimd
            # queue + DVE centering.  Staging ring overlaps DMA/compute.
            ystg = [stg.tile([128, HW], F32, tag="stage", name=f"ys{g}")
                    for g in range(2)]
            for g in range(2):
                nc.sync.dma_start(
                    out=ystg[g][:, :],
                    in_=ys.rearrange("(g p) j -> g p j", p=128)[g])
            xstg = [stg.tile([128, HW], F32, tag="stage", name=f"xs{g}")
                    for g in range(2)]
            for g in range(2):
                nc.gpsimd.dma_start(
                    out=xstg[g][:, :],
                    in_=xs.rearrange("(g p) j -> g p j", p=128)[g])
            for g in range(2):
                nc.scalar.activation(
                    out=yn[:, g, :], in_=ystg[g][:, :],
                    func=ACTF.Identity, bias=negymu[:, g : g + 1], scale=1.0)
            for g in range(2):
                nc.vector.tensor_scalar(
                    out=xc[:, g, :], in0=xstg[g][:, :],
                    scalar1=ymu_sb[:, g : g + 1], scalar2=None,
                    op0=ALU.subtract)

            # ---------------- sumsq rows -> invy/invx ---------------------
            nrow = sm.tile([1, HW], F32, tag="nrow")

            def sumsq_rows(src, scr, use_act):
                sqs = []
                for g in range(2):
                    sq = sqp.tile([128, HW], BF16, tag="sqt")
                    if use_act:
                        nc.scalar.activation(
                            out=sq[:, :], in_=src[:, g, :],
                            func=ACTF.Square)
                    else:
                        nc.vector.tensor_tensor(
                            out=sq[:, :], in0=src[:, g, :],
                            in1=src[:, g, :], op=ALU.mult)
                    sqs.append(sq)
                for h in range(HW // TW):
                    pr = mmq.tile([1, TW], F32, tag="pq",
                                  name=f"pr_{scr.name}_{h}")
                    for g in range(2):
                        for s in range(NS):
                            j0 = TW * h + 512 * s
                            nc.tensor.matmul(
                                pr[:, 512 * s : 512 * (s + 1)],
                                ones_col[:, :],
                                sqs[g][:, j0 : j0 + 512],
                                start=(g == 0), stop=(g == 1))
                    nc.scalar.activation(
                        out=nrow[0:1, TW * h : TW * (h + 1)],
                        in_=pr[0:1, :], func=ACTF.Copy)
                nc.sync.dma_start(out=scr[0, :], in_=nrow[0:1, :])

            sumsq_rows(yn, scr_ny, use_act=True)
            nsqy = sm.tile([128, NB], F32, tag="nsqy")
            nc.sync.dma_start(
                out=nsqy[:, :], in_=scr_ny[0, :].rearrange("(r p) -> p r", p=128))
            normy = sm.tile([128, NB], F32, tag="normy")
            nc.scalar.activation(out=normy[:, :], in_=nsqy[:, :], func=ACTF.Sqrt)
            invy = sm.tile([128, NB], F32, tag="invy")
            nc.vector.reciprocal(invy[:, :], normy[:, :])
            nc.sync.dma_start(
                out=scr_y[:, :].rearrange("r p -> p r"), in_=invy[:, :])
            invybc = bc3.tile([128, HW], F32, tag="bcast")
            bcast_src_y = bass_mod.AP(
                tensor=scr_y[:, :].tensor, offset=0, ap=[[0, 128], [1, HW]])
            nc.sync.dma_start(out=invybc[:, :], in_=bcast_src_y)

            # y_n = y_c * invy (in place, f32r)
            for g in range(2):
                nc.vector.tensor_tensor(
                    out=yn[:, g, :], in0=yn[:, g, :],
                    in1=invybc[:, :], op=ALU.mult)

            # x norms (overlaps pass1 head; only needed by the stats chain)
            sumsq_rows(xc, scr_nx, use_act=True)
            nsqx = sm.tile([128, NB], F32, tag="nsqx")
            nc.sync.dma_start(
                out=nsqx[:, :], in_=scr_nx[0, :].rearrange("(r p) -> p r", p=128))
            normx = sm.tile([128, NB], F32, tag="normx")
            nc.scalar.activation(out=normx[:, :], in_=nsqx[:, :], func=ACTF.Sqrt)
            invx = sm.tile([128, NB], F32, tag="invx")
            nc.vector.reciprocal(invx[:, :], normx[:, :])
            invx2 = sm.tile([128, NB], F32, tag="invx2")     # 2*invx
            nc.vector.tensor_scalar(
                out=invx2[:, :], in0=invx[:, :], scalar1=2.0,
                scalar2=None, op0=ALU.mult)
            ninvx = sm.tile([128, NB], F32, tag="ninvx")     # -invx
            nc.vector.tensor_scalar(
                out=ninvx[:, :], in0=invx[:, :], scalar1=-1.0,
                scalar2=None, op0=ALU.mult)

            # ---------------- PASS 1: row max + Z -------------------------
            gacc = sm.tile([128, NB * NTB], F32, tag="gacc")
            zacc = sm.tile([128, NB * NTB], F32, tag="zacc")
            gmaxc = sm.tile([128, NB], F32, tag="gmaxc")
            dummy1 = sm.tile([128, 1], F32, tag="dummy1")
            if USE_FMAX:
                zerobc = sm.tile([128, TW], F32, tag="zerobc")
                nc.vector.memset(zerobc[:, :], 0.0)
            reccol = sm.tile([128, NB], F32, tag="reccol")
            tmpc = sm.tile([128, NB], F32, tag="tmpc")
            ab2 = sm.tile([128, 2 * NB], F32, tag="ab2")  # alpha | b2
            bcol = sm.tile([128, NB], F32, tag="bcol")
            zsum = sm.tile([128, NB], F32, tag="zsum")
            lnz = sm.tile([128, NB], F32, tag="lnz")

            for r in range(NB):
                pqs = [mmq.tile([128, TW], F32, tag="pq", name=f"p1_{r}_{t}")
                       for t in range(NTB)]
                for t in range(NTB):
                    for g in range(2):
                        for s in range(NS):
                            j0 = TW * t + 512 * s
                            nc.tensor.matmul(
                                pqs[t][:, 512 * s : 512 * (s + 1)],
                                xc[:, g, 128 * r : 128 * (r + 1)],
                                yn[:, g, j0 : j0 + 512],
                                start=(g == 0), stop=(g == 1))
                    if USE_FMAX:
                        nc.vector._custom_dve(
                            _AMR,
                            out=dummy1[:, 0:1].broadcast_to((128, TW)),
                            in0=pqs[t][:, :],
                            in1=zerobc[:, :],
                            s1=-1e30,
                            accum_out=gacc[:, NTB * r + t : NTB * r + t + 1])
                    else:
                        nc.vector.reduce_max(
                            gacc[:, NTB * r + t : NTB * r + t + 1],
                            pqs[t][:, :], axis=mybir.AxisListType.X)
                # all-DVE row stats chain (back-to-back, same engine)
                nc.vector.reduce_max(
                    gmaxc[:, r : r + 1],
                    gacc[:, NTB * r : NTB * (r + 1)],
                    axis=mybir.AxisListType.X)
                nc.vector.tensor_scalar(
                    out=tmpc[:, r : r + 1], in0=gmaxc[:, r : r + 1],
                    scalar1=ninvx[:, r : r + 1], scalar2=float(1.0 + EPS),
                    op0=ALU.mult, op1=ALU.add)
                nc.vector.reciprocal(reccol[:, r : r + 1], tmpc[:, r : r + 1])
                nc.vector.tensor_scalar(
                    out=ab2[:, r : r + 1], in0=reccol[:, r : r + 1],
                    scalar1=invx2[:, r : r + 1], scalar2=None, op0=ALU.mult)
                nc.vector.tensor_scalar(
                    out=bcol[:, r : r + 1], in0=reccol[:, r : r + 1],
                    scalar1=-2.0, scalar2=2.0, op0=ALU.mult, op1=ALU.add)
                for t in range(NTB):
                    nc.scalar.activation(
                        out=pqs[t][:, :], in_=pqs[t][:, :], func=ACTF.Exp,
                        bias=bcol[:, r : r + 1],
                        scale=ab2[:, r : r + 1],
                        accum_out=zacc[:, NTB * r + t : NTB * r + t + 1])

            # ---------------- interlude: b2 = b - lnZ; broadcasts ---------
            nc.vector.reduce_sum(
                zsum[:, :],
                zacc[:, :].rearrange("p (r q) -> p r q", q=NTB),
                axis=mybir.AxisListType.X)
            nc.scalar.activation(out=lnz[:, :], in_=zsum[:, :], func=ACTF.Ln)
            nc.vector.tensor_tensor(
                out=ab2[:, NB : 2 * NB], in0=bcol[:, :], in1=lnz[:, :],
                op=ALU.subtract)

            # alpha -> bcast via SP queue; b2 -> row via gpsimd queue
            nc.sync.dma_start(
                out=scr_a[:, :].rearrange("r p -> p r"), in_=ab2[:, 0:NB])
            nc.gpsimd.dma_start(
                out=scr_b[:, :].rearrange("r p -> p r"),
                in_=ab2[:, NB : 2 * NB])
            abc = bc3.tile([128, HW], F32, tag="bcast")
            bcast_src_a = bass_mod.AP(
                tensor=scr_a[:, :].tensor, offset=0, ap=[[0, 128], [1, HW]])
            nc.sync.dma_start(out=abc[:, :], in_=bcast_src_a)
            if USE_DVEMAX:
                b2bc = bc3.tile([128, HW], F32, tag="bcast2")
                bcast_src_b = bass_mod.AP(
                    tensor=scr_b[:, :].tensor, offset=0,
                    ap=[[0, 128], [1, HW]])
                nc.gpsimd.dma_start(out=b2bc[:, :], in_=bcast_src_b)
                b2row = None
            else:
                nc.gpsimd.dma_start(
                    out=nrow[0:1, :],
                    in_=scr_b[:, :].rearrange("r p -> (r p)"))
                b2row = sm.tile([1, HW], F32R, tag="b2row")
                nc.vector.tensor_scalar(
                    out=b2row[:, :], in0=nrow[0:1, :], scalar1=1.0,
                    scalar2=None, op0=ALU.mult)

            # x2 = x_c * alpha (in place, f32r), chunked so pass2 can start
            XCH = 1024
            for j0 in range(0, HW, XCH):
                for g in range(2):
                    nc.vector.tensor_tensor(
                        out=xc[:, g, j0 : j0 + XCH],
                        in0=xc[:, g, j0 : j0 + XCH],
                        in1=abc[:, j0 : j0 + XCH], op=ALU.mult)

            # optional fp8 copies for pass2 DoubleRow matmuls
            if USE_FP8:
                x2f8 = stg.tile([128, 2, HW], FP8, tag="f8", name="x2f8")
                ynf8 = stg.tile([128, 2, HW], FP8, tag="f8", name="ynf8")
                for g in range(2):
                    nc.vector.tensor_scalar(
                        out=x2f8[:, g, :], in0=xc[:, g, :],
                        scalar1=1.0, scalar2=None, op0=ALU.mult)
                    nc.scalar.activation(
                        out=ynf8[:, g, :], in_=yn[:, g, :],
                        func=ACTF.Copy)

            # ---------------- PASS 2: bias seed + col max -----------------
            macc = sm.tile([128, NB * NTB], F32, tag="macc")
            dummy = sm.tile([128, 1], F32, tag="dummy")
            for rb in range(NB):
                pqs = [mmq.tile([128, TW], F32, tag="pq", name=f"p2_{rb}_{t}")
                       for t in range(NTB)]
                for t in range(NTB):
                    if not USE_DVEMAX:
                        for s in range(NS):
                            j0 = TW * t + 512 * s
                            nc.tensor.matmul(
                                pqs[t][:, 512 * s : 512 * (s + 1)],
                                ones_row[:, :],
                                b2row[:, j0 : j0 + 512],
                                start=True, stop=False)
                    if USE_FP8:
                        for s in range(NS):
                            j0 = TW * t + 512 * s
                            nc.tensor.matmul(
                                pqs[t][:, 512 * s : 512 * (s + 1)],
                                ynf8[:, :, 128 * rb : 128 * (rb + 1)],
                                x2f8[:, :, j0 : j0 + 512],
                                start=False, stop=True,
                                perf_mode=mybir.MatmulPerfMode.DoubleRow)
                    else:
                        for g in range(2):
                            for s in range(NS):
                                j0 = TW * t + 512 * s
                                nc.tensor.matmul(
                                    pqs[t][:, 512 * s : 512 * (s + 1)],
                                    yn[:, g, 128 * rb : 128 * (rb + 1)],
                                    xc[:, g, j0 : j0 + 512],
                                    start=(USE_DVEMAX and g == 0),
                                    stop=(g == 1))
                    if USE_DVEMAX:
                        nc.vector._custom_dve(
                            _AMR,
                            out=dummy[:, 0:1].broadcast_to((128, TW)),
                            in0=pqs[t][:, :],
                            in1=b2bc[:, TW * t : TW * (t + 1)],
                            s1=-1e30,
                            accum_out=macc[:, NTB * rb + t : NTB * rb + t + 1])
                    elif USE_FMAX:
                        nc.vector._custom_dve(
                            _AMR,
                            out=dummy[:, 0:1].broadcast_to((128, TW)),
                            in0=pqs[t][:, :],
                            in1=zerobc[:, :],
                            s1=-1e30,
                            accum_out=macc[:, NTB * rb + t : NTB * rb + t + 1])
                    else:
                        nc.vector.reduce_max(
                            macc[:, NTB * rb + t : NTB * rb + t + 1],
                            pqs[t][:, :], axis=mybir.AxisListType.X)

            # ---------------- final ---------------------------------------
            mcol = sm.tile([128, NB], F32, tag="mcol")
            nc.vector.reduce_max(
                mcol[:, :],
                macc[:, :].rearrange("p (r q) -> p r q", q=NTB),
                axis=mybir.AxisListType.X)
            expm = sm.tile([128, NB], F32, tag="expm")
            csum = sm.tile([128, 1], F32, tag="csum")
            nc.scalar.activation(
                out=expm[:, :], in_=mcol[:, :], func=ACTF.Exp,
                accum_out=csum[:, :])
            tot = sm.tile([128, 1], F32, tag="tot")
            nc.gpsimd.partition_all_reduce(
                tot[:, :], csum[:, :], channels=128,
                reduce_op=bass_isa.ReduceOp.add)
            res = sm.tile([1, 1], F32, tag="res")
            nc.vector.tensor_scalar(
                out=res[:, :], in0=tot[0:1, :], scalar1=float(1.0 / HW),
                scalar2=None, op0=ALU.mult)
            nc.sync.dma_start(out=out[:, :], in_=res[:, :])
    nc.compile()
    return nc


def _get_nc():
    if "nc" not in _cached:
        _cached["nc"] = _build()
    return _cached["nc"]


def run_device(x, y, trace=False):
    """x, y: (N, C, H, W) float32. Returns (ccx (N,), BassKernelResults)."""
    x = np.ascontiguousarray(np.asarray(x, dtype=np.float32))
    y = np.ascontiguousarray(np.asarray(y, dtype=np.float32))
    ymu = y.mean(axis=(0, 2, 3), dtype=np.float64).astype(np.float32)  # (C,)
    ymu_arr = np.ascontiguousarray(ymu.reshape(2, 128).T)  # (128, 2)
    in_maps = []
    for n in range(N):
        in_maps.append({
            "xs": np.ascontiguousarray(x[n].reshape(C, HW)),
            "ys": np.ascontiguousarray(y[n].reshape(C, HW)),
            "ymu": ymu_arr,
        })
    nc = _get_nc()
    res = run_bass_kernel_spmd(nc, in_maps, core_ids=list(range(N)), trace=trace)
    ccx = np.array([res.results[n]["out"][0, 0] for n in range(N)], dtype=np.float32)
    return ccx, res


def kernel(x, y):
    ccx, _ = run_device(x, y)
    loss = float(np.mean(-np.log(ccx.astype(np.float64) + EPS)))
    return np.float32(loss)


if __name__ == "__main__":
    rng = np.random.default_rng(0)
    x = rng.standard_normal((N, C, H, W), dtype=np.float32)
    y = rng.standard_normal((N, C, H, W), dtype=np.float32)
    print("loss:", kernel(x, y))
